# revision 23
# baseline (speedup 1.0000x reference)
"""Trainium2 Bass kernel for Euler-integrated Kuramoto dynamics.

    dtheta_i/dt = omega_i + sum_j K[i,j] * sin(theta_j - theta_i)

Strategy (8 NeuronCores, SPMD):
  sin(theta_j - theta_i) = sin(theta_j)cos(theta_i) - cos(theta_j)sin(theta_i)
so the per-step coupling reduction is two matvecs against K:
  coupling = cos(theta) * (K @ sin(theta)) - sin(theta) * (K @ cos(theta))

K is sharded row-wise: core c owns rows [512c, 512c+512). The shard is
staged as lhsT (K[rows,:].T scaled by dt/2pi, shape (4096, 512)) in fp16
and stays resident in SBUF for all 50 steps (4 MB/core) — the matvec runs
with K as the stationary operand (fp16 => fast-weight-load) and a tiny
(128, 2) moving sin/cos operand. Each step every core updates its own 512
phases, then the sin/cos of the updated shard (fp32, 4 KB) is AllGathered
so the next step's matvec has the full sin/cos vectors.

Phase state is kept in TURNS (theta/2pi), pre-duplicated as
tau8[p, 2a+h] = theta[c*512 + a*128 + p]/2pi + 0.25*h, so one [128,8]
Sin activation yields sin (h=0) and cos (h=1) after the
f = u - round(u) range reduction ((u+1.5*2^23)-1.5*2^23 rounding trick).
dt/(2pi) is folded into K host-side and omega*dt/2pi is a resident tile,
so the whole per-step update is 7 vector ops + 1 activation.

DMA-layout: the AllGather bounce buffers are fp32 pairs ordered so both
bounce DMAs are per-partition contiguous (128 descriptors of 32B out /
256B in, vs thousands of 2-byte descriptors in the naive layout, whose
HBM read-modify-write completion tail cost ~30 us/step). The price is a
static permutation of K's columns, baked host-side into kt:
contraction tile t, partition p of the matmul holds global element
  gg(t,p) = 512*(p>>4) + 128*(t&3) + 8*(p&15) + (t>>2)
which makes the gathered buffer read exactly cout[(p f) -> p f].
"""

import os as _os

import numpy as np

N = 4096
M = 8  # cores
S = N // M  # 512 phases per core
NT = N // 128  # 32 contraction k-tiles
IT = S // 128  # 4 output i-tiles per core

N_STEPS = int(_os.environ.get("KUR_STEPS", "50"))
N_DUMMIES = int(_os.environ.get("KUR_DUMMIES", "0"))
DT = 0.01
PI = 3.141592653589793
TWO_PI = 2.0 * PI
INV2PI = 1.0 / TWO_PI

TRACE = False
LAST_RESULTS = None

_compiled_nc = None


def _build(n_steps=None, n_dummies=None):
    import concourse.bass as bass  # noqa: F401
    import concourse.tile as tile
    from concourse import bacc, mybir

    if n_steps is None:
        n_steps = N_STEPS
    if n_dummies is None:
        n_dummies = N_DUMMIES

    f32 = mybir.dt.float32
    f16 = mybir.dt.float16
    AF = mybir.ActivationFunctionType
    OP = mybir.AluOpType

    nc = bacc.Bacc(
        "TRN2",
        target_bir_lowering=False,
        debug=False,
        enable_asserts=False,
        num_devices=M,
    )
    kt = nc.dram_tensor("kt", [N, S], f16, kind="ExternalInput").ap()
    sc0 = nc.dram_tensor("sc0", [128 * 64], f32, kind="ExternalInput").ap()
    t80 = nc.dram_tensor("t80", [128 * 8], f32, kind="ExternalInput").ap()
    om8i = nc.dram_tensor("om8", [128 * 8], f32, kind="ExternalInput").ap()
    sco0 = nc.dram_tensor("sco0", [128 * 8], f32, kind="ExternalInput").ap()
    th_out = nc.dram_tensor("th_out", [S], f32, kind="ExternalOutput").ap()

    # (u + BIG) - BIG == round-to-nearest-integer(u) in fp32; the 1.5x
    # keeps u + BIG inside [2^23, 2^24) (ulp exactly 1) for negative u too
    BIG = 1.5 * 2.0**23

    with tile.TileContext(nc) as tc:
        with (
            tc.tile_pool(name="pers", bufs=1) as pers,
            tc.tile_pool(name="psum", bufs=2, space="PSUM") as psum_pool,
            tc.tile_pool(name="psd", bufs=1, space="PSUM") as psum_dum,
            tc.tile_pool(name="work", bufs=2) as work,
            tc.tile_pool(name="dram", bufs=1, space="DRAM") as dram,
        ):
            KT = pers.tile([128, NT * S], f16)  # k-tile t at cols [t*512,(t+1)*512)
            SC32 = pers.tile([128, 64], f32)  # gathered sincos, col 2t+h
            SCo32 = pers.tile([128, 8], f32)  # own sincos, col 2a+h
            TAU8 = pers.tile([128, 8], f32)  # theta/2pi + 0.25h
            TOM8 = pers.tile([128, 8], f32)  # TAU8 + OM8 (next-step base)
            OM8 = pers.tile([128, 8], f32)  # dt*omega/2pi, duplicated pairs

            cin = dram.tile([128 * 8], f32, tag="cin")
            cout = dram.tile([128 * 64], f32, tag="cout")

            # --- preamble: K resident load + initial state ---
            for t in range(NT):
                nc.sync.dma_start(KT[:, t * S : (t + 1) * S], kt[t * 128 : (t + 1) * 128, :])
            nc.sync.dma_start(SC32[:], sc0.rearrange("(p f) -> p f", p=128))
            nc.sync.dma_start(TAU8[:], t80.rearrange("(p f) -> p f", p=128))
            nc.sync.dma_start(OM8[:], om8i.rearrange("(p f) -> p f", p=128))
            nc.sync.dma_start(SCo32[:], sco0.rearrange("(p f) -> p f", p=128))
            nc.vector.tensor_tensor(TOM8[:], TAU8[:], OM8[:], OP.add)

            for s in range(n_steps):
                SCh = work.tile([128, 64], f16, tag="sch")
                nc.vector.tensor_copy(SCh[:], SC32[:])

                ps = psum_pool.tile([128, 2 * IT], f32)
                for it in range(IT):
                    base = it * 128
                    for t in range(NT):
                        nc.tensor.matmul(
                            ps[:, 2 * it : 2 * it + 2],
                            lhsT=KT[:, t * S + base : t * S + base + 128],
                            rhs=SCh[:, 2 * t : 2 * t + 2],  # {sin_t, cos_t}
                            start=(t == 0),
                            stop=(t == NT - 1),
                        )

                # e = (cos_own*(K~@s) - sin_own*(K~@c))  [in turns, dt/2pi folded]
                prs = work.tile([128, IT], f32, tag="prs")
                prc = work.tile([128, IT], f32, tag="prc")
                e = work.tile([128, IT], f32, tag="e")
                nc.vector.tensor_tensor(prs[:], SCo32[:, 1::2], ps[:, 0::2], OP.mult)
                nc.vector.tensor_tensor(prc[:], SCo32[:, 0::2], ps[:, 1::2], OP.mult)
                nc.vector.tensor_tensor(e[:], prs[:], prc[:], OP.subtract)
                # tau' = tau + dt*omega/2pi + e  (both quarter-offset halves)
                nc.vector.tensor_tensor(TAU8[:, 0::2], TOM8[:, 0::2], e[:], OP.add)
                nc.vector.tensor_tensor(TAU8[:, 1::2], TOM8[:, 1::2], e[:], OP.add)

                if s == n_steps - 1:
                    break

                # sincos of updated own shard: f = tau - round(tau); Sin(2pi f)
                w8 = work.tile([128, 8], f32, tag="w8")
                f8 = work.tile([128, 8], f32, tag="f8")
                nc.vector.tensor_scalar(w8[:], TAU8[:], BIG, BIG, OP.add, OP.subtract)
                nc.vector.tensor_tensor(f8[:], TAU8[:], w8[:], OP.subtract)
                nc.scalar.activation(SCo32[:], f8[:], AF.Sin, scale=TWO_PI)
                # off the critical path: next-step base + the f16 anchor that
                # orders this step's PE keep-warm dummies after the combine
                nc.vector.tensor_tensor(TOM8[:], TAU8[:], OM8[:], OP.add)
                if n_dummies:
                    anc = work.tile([128, 8], f16, tag="anc")
                    nc.vector.tensor_copy(anc[:], f8[:])

                # bounce out (128 x 32B contiguous), AllGather, bounce in
                nc.scalar.dma_start(cin.rearrange("(p f) -> p f", p=128), SCo32[:])
                nc.gpsimd.collective_compute(
                    "AllGather",
                    OP.bypass,
                    replica_groups=[list(range(M))],
                    ins=[cin.opt()],
                    outs=[cout.opt()],
                )
                if n_dummies:
                    # keep the PE's HAM activity monitor busy through the
                    # collective gap so matmuls stay at 2.4 GHz; anchored to
                    # this step's post-combine anchor so the scheduler can't
                    # float them ahead of the real matmuls
                    psd = psum_dum.tile([8, 512], f32, tag="psd")
                    for _ in range(n_dummies):
                        nc.tensor.matmul(
                            psd[:, :],
                            lhsT=anc[:, 0:8],
                            rhs=KT[:, 0:512],
                            start=True,
                            stop=True,
                        )
                nc.sync.dma_start(SC32[:], cout.rearrange("(p f) -> p f", p=128))

            tho = work.tile([128, IT], f32, tag="tho")
            nc.vector.tensor_scalar(tho[:], TAU8[:, 0::2], TWO_PI, 0.0, OP.mult, OP.add)
            nc.sync.dma_start(th_out.rearrange("(p a) -> p a", p=128), tho[:])

    nc.compile()
    return nc


def _get_nc():
    global _compiled_nc
    if _compiled_nc is None:
        _compiled_nc = _build()
    return _compiled_nc


def kernel(phases, K, omegas):
    global LAST_RESULTS
    from concourse import bass_utils

    phases = np.ascontiguousarray(np.asarray(phases, dtype=np.float32))
    K = np.asarray(K, dtype=np.float32)
    omegas = np.asarray(omegas, dtype=np.float32)

    Ks = K * np.float32(DT * INV2PI)
    p = np.arange(128)
    t = np.arange(NT)
    # gg[t, p]: global element index at contraction tile t, partition p
    gg = (
        512 * (p[None, :] >> 4)
        + 128 * (t[:, None] & 3)
        + 8 * (p[None, :] & 15)
        + (t[:, None] >> 2)
    )
    perm = gg.reshape(-1)  # j = t*128 + p

    sin_g = np.sin(phases).astype(np.float32)
    cos_g = np.cos(phases).astype(np.float32)
    sc0 = np.empty((128, 64), np.float32)
    sc0[:, 0::2] = sin_g[gg].T  # [128, 32]
    sc0[:, 1::2] = cos_g[gg].T

    nc = _get_nc()
    in_maps = []
    for c in range(M):
        sl = slice(c * S, (c + 1) * S)
        th = phases[sl].reshape(IT, 128)  # [a, p]
        om = (omegas[sl] * np.float32(DT * INV2PI)).reshape(IT, 128)
        t80 = np.empty((128, 8), np.float32)
        om8 = np.empty((128, 8), np.float32)
        for a in range(IT):
            t80[:, 2 * a] = th[a] * np.float32(INV2PI)
            t80[:, 2 * a + 1] = th[a] * np.float32(INV2PI) + np.float32(0.25)
            om8[:, 2 * a] = om[a]
            om8[:, 2 * a + 1] = om[a]
        sco0 = np.sin(TWO_PI * t80.astype(np.float64)).astype(np.float32)
        in_maps.append(
            {
                "kt": np.ascontiguousarray(Ks[sl, :][:, perm].T).astype(np.float16),
                "sc0": np.ascontiguousarray(sc0.reshape(-1)),
                "t80": np.ascontiguousarray(t80.reshape(-1)),
                "om8": np.ascontiguousarray(om8.reshape(-1)),
                "sco0": np.ascontiguousarray(sco0.reshape(-1)),
            }
        )
    res = bass_utils.run_bass_kernel_spmd(
        nc, in_maps, core_ids=list(range(M)), trace=TRACE
    )
    LAST_RESULTS = res
    out = np.concatenate(
        [res.results[c]["th_out"].reshape(128, IT).T.reshape(-1) for c in range(M)]
    )
    return out.astype(np.float32)


# revision 25
# speedup vs baseline: 1.0126x; 1.0126x over previous
"""Trainium2 Bass kernel for Euler-integrated Kuramoto dynamics.

    dtheta_i/dt = omega_i + sum_j K[i,j] * sin(theta_j - theta_i)

Strategy (8 NeuronCores, SPMD):
  sin(theta_j - theta_i) = sin(theta_j)cos(theta_i) - cos(theta_j)sin(theta_i)
so the per-step coupling reduction is two matvecs against K:
  coupling = cos(theta) * (K @ sin(theta)) - sin(theta) * (K @ cos(theta))

K is sharded row-wise: core c owns rows [512c, 512c+512). The shard is
staged as lhsT (K[rows,:].T scaled by dt/2pi, shape (4096, 512)) in fp16
and stays resident in SBUF for all 50 steps (4 MB/core) — the matvec runs
with K as the stationary operand (fp16 => fast-weight-load) and a tiny
(128, 2) moving sin/cos operand. Each step every core updates its own 512
phases, then the sin/cos of the updated shard (fp32, 4 KB) is AllGathered
so the next step's matvec has the full sin/cos vectors.

Phase state is kept in TURNS (theta/2pi), pre-duplicated as
tau8[p, 2a+h] = theta[c*512 + a*128 + p]/2pi + 0.25*h, so one [128,8]
Sin activation yields sin (h=0) and cos (h=1) after the
f = u - round(u) range reduction ((u+1.5*2^23)-1.5*2^23 rounding trick).
dt/(2pi) is folded into K host-side and omega*dt/2pi is a resident tile,
so the whole per-step update is 7 vector ops + 1 activation.

DMA-layout: the AllGather bounce buffers are fp32 pairs ordered so both
bounce DMAs are per-partition contiguous (128 descriptors of 32B out /
256B in, vs thousands of 2-byte descriptors in the naive layout, whose
HBM read-modify-write completion tail cost ~30 us/step). The price is a
static permutation of K's columns, baked host-side into kt:
contraction tile t, partition p of the matmul holds global element
  gg(t,p) = 512*(p>>4) + 128*(t&3) + 8*(p&15) + (t>>2)
which makes the gathered buffer read exactly cout[(p f) -> p f].
"""

import os as _os

import numpy as np

N = 4096
M = 8  # cores
S = N // M  # 512 phases per core
NT = N // 128  # 32 contraction k-tiles
IT = S // 128  # 4 output i-tiles per core

N_STEPS = int(_os.environ.get("KUR_STEPS", "50"))
N_DUMMIES = int(_os.environ.get("KUR_DUMMIES", "0"))
DT = 0.01
PI = 3.141592653589793
TWO_PI = 2.0 * PI
INV2PI = 1.0 / TWO_PI

TRACE = False
LAST_RESULTS = None

_compiled_nc = None


def _build(n_steps=None, n_dummies=None):
    import concourse.bass as bass  # noqa: F401
    import concourse.tile as tile
    from concourse import bacc, mybir

    if n_steps is None:
        n_steps = N_STEPS
    if n_dummies is None:
        n_dummies = N_DUMMIES

    f32 = mybir.dt.float32
    f16 = mybir.dt.float16
    AF = mybir.ActivationFunctionType
    OP = mybir.AluOpType

    nc = bacc.Bacc(
        "TRN2",
        target_bir_lowering=False,
        debug=False,
        enable_asserts=False,
        num_devices=M,
    )
    kt = nc.dram_tensor("kt", [N, S], f16, kind="ExternalInput").ap()
    sc0 = nc.dram_tensor("sc0", [128 * 64], f32, kind="ExternalInput").ap()
    t80 = nc.dram_tensor("t80", [128 * 8], f32, kind="ExternalInput").ap()
    om8i = nc.dram_tensor("om8", [128 * 8], f32, kind="ExternalInput").ap()
    sco0 = nc.dram_tensor("sco0", [128 * 8], f32, kind="ExternalInput").ap()
    th_out = nc.dram_tensor("th_out", [S], f32, kind="ExternalOutput").ap()

    # (u + BIG) - BIG == round-to-nearest-integer(u) in fp32; the 1.5x
    # keeps u + BIG inside [2^23, 2^24) (ulp exactly 1) for negative u too
    BIG = 1.5 * 2.0**23

    with tile.TileContext(nc) as tc:
        with (
            tc.tile_pool(name="pers", bufs=1) as pers,
            tc.tile_pool(name="psum", bufs=2, space="PSUM") as psum_pool,
            tc.tile_pool(name="psd", bufs=1, space="PSUM") as psum_dum,
            tc.tile_pool(name="work", bufs=2) as work,
            tc.tile_pool(name="dram", bufs=1, space="DRAM") as dram,
        ):
            KT = pers.tile([128, NT * S], f16)  # k-tile t at cols [t*512,(t+1)*512)
            SC32 = pers.tile([128, 64], f32)  # gathered sincos, col 2t+h
            SCo32 = pers.tile([128, 8], f32)  # own sincos, col 2a+h
            TAU8 = pers.tile([128, 8], f32)  # theta/2pi + 0.25h
            TOM8 = pers.tile([128, 8], f32)  # TAU8 + OM8 (next-step base)
            OM8 = pers.tile([128, 8], f32)  # dt*omega/2pi, duplicated pairs

            cin = dram.tile([128 * 8], f32, tag="cin")
            cout = dram.tile([128 * 64], f32, tag="cout")

            # --- preamble: K resident load + initial state ---
            for t in range(NT):
                nc.sync.dma_start(KT[:, t * S : (t + 1) * S], kt[t * 128 : (t + 1) * 128, :])
            nc.sync.dma_start(SC32[:], sc0.rearrange("(p f) -> p f", p=128))
            nc.sync.dma_start(TAU8[:], t80.rearrange("(p f) -> p f", p=128))
            nc.sync.dma_start(OM8[:], om8i.rearrange("(p f) -> p f", p=128))
            nc.sync.dma_start(SCo32[:], sco0.rearrange("(p f) -> p f", p=128))
            nc.vector.tensor_tensor(TOM8[:], TAU8[:], OM8[:], OP.add)

            for s in range(n_steps):
                SCh = work.tile([128, 64], f16, tag="sch")
                nc.vector.tensor_copy(SCh[:], SC32[:])

                ps = psum_pool.tile([128, 2 * IT], f32)
                for it in range(IT):
                    base = it * 128
                    for t in range(NT):
                        nc.tensor.matmul(
                            ps[:, 2 * it : 2 * it + 2],
                            lhsT=KT[:, t * S + base : t * S + base + 128],
                            rhs=SCh[:, 2 * t : 2 * t + 2],  # {sin_t, cos_t}
                            start=(t == 0),
                            stop=(t == NT - 1),
                        )

                # e = (cos_own*(K~@s) - sin_own*(K~@c))  [in turns, dt/2pi folded]
                prs = work.tile([128, IT], f32, tag="prs")
                prc = work.tile([128, IT], f32, tag="prc")
                e = work.tile([128, IT], f32, tag="e")
                nc.vector.tensor_tensor(prs[:], SCo32[:, 1::2], ps[:, 0::2], OP.mult)
                nc.vector.tensor_tensor(prc[:], SCo32[:, 0::2], ps[:, 1::2], OP.mult)
                nc.vector.tensor_tensor(e[:], prs[:], prc[:], OP.subtract)
                # tau' = tau + dt*omega/2pi + e  (both quarter-offset halves)
                nc.vector.tensor_tensor(TAU8[:, 0::2], TOM8[:, 0::2], e[:], OP.add)
                nc.vector.tensor_tensor(TAU8[:, 1::2], TOM8[:, 1::2], e[:], OP.add)

                if s == n_steps - 1:
                    break

                # sincos of updated own shard: f = tau - round(tau); Sin(2pi f)
                w8 = work.tile([128, 8], f32, tag="w8")
                f8 = work.tile([128, 8], f32, tag="f8")
                nc.vector.tensor_scalar(w8[:], TAU8[:], BIG, BIG, OP.add, OP.subtract)
                nc.vector.tensor_tensor(f8[:], TAU8[:], w8[:], OP.subtract)
                nc.scalar.activation(SCo32[:], f8[:], AF.Sin, scale=TWO_PI)
                # off the critical path: next-step base + the f16 anchor that
                # orders this step's PE keep-warm dummies after the combine
                nc.vector.tensor_tensor(TOM8[:], TAU8[:], OM8[:], OP.add)
                if n_dummies:
                    anc = work.tile([128, 8], f16, tag="anc")
                    nc.vector.tensor_copy(anc[:], f8[:])

                # bounce out (128 x 32B contiguous), AllGather, bounce in
                nc.scalar.dma_start(cin.rearrange("(p f) -> p f", p=128), SCo32[:])
                nc.gpsimd.collective_compute(
                    "AllGather",
                    OP.bypass,
                    replica_groups=[list(range(M))],
                    ins=[cin.opt()],
                    outs=[cout.opt()],
                )
                if n_dummies:
                    # keep the PE's HAM activity monitor busy through the
                    # collective gap so matmuls stay at 2.4 GHz; anchored to
                    # this step's post-combine anchor so the scheduler can't
                    # float them ahead of the real matmuls
                    psd = psum_dum.tile([8, 512], f32, tag="psd")
                    for _ in range(n_dummies):
                        nc.tensor.matmul(
                            psd[:, :],
                            lhsT=anc[:, 0:8],
                            rhs=KT[:, 0:512],
                            start=True,
                            stop=True,
                        )
                nc.sync.dma_start(SC32[:], cout.rearrange("(p f) -> p f", p=128))

            tho = work.tile([128, IT], f32, tag="tho")
            nc.vector.tensor_scalar(tho[:], TAU8[:, 0::2], TWO_PI, 0.0, OP.mult, OP.add)
            nc.sync.dma_start(th_out.rearrange("(p a) -> p a", p=128), tho[:])

    nc.compile()
    return nc


def _get_nc():
    global _compiled_nc
    if _compiled_nc is None:
        _compiled_nc = _build()
    return _compiled_nc


def kernel(phases, K, omegas):
    global LAST_RESULTS
    from concourse import bass_utils

    phases = np.ascontiguousarray(np.asarray(phases, dtype=np.float32))
    K = np.asarray(K, dtype=np.float32)
    omegas = np.asarray(omegas, dtype=np.float32)

    Ks = K * np.float32(DT * INV2PI)
    p = np.arange(128)
    t = np.arange(NT)
    # gg[t, p]: global element index at contraction tile t, partition p
    gg = (
        512 * (p[None, :] >> 4)
        + 128 * (t[:, None] & 3)
        + 8 * (p[None, :] & 15)
        + (t[:, None] >> 2)
    )
    perm = gg.reshape(-1)  # j = t*128 + p

    sin_g = np.sin(phases).astype(np.float32)
    cos_g = np.cos(phases).astype(np.float32)
    sc0 = np.empty((128, 64), np.float32)
    sc0[:, 0::2] = sin_g[gg].T  # [128, 32]
    sc0[:, 1::2] = cos_g[gg].T

    nc = _get_nc()
    in_maps = []
    for c in range(M):
        sl = slice(c * S, (c + 1) * S)
        th = phases[sl].reshape(IT, 128)  # [a, p]
        om = (omegas[sl] * np.float32(DT * INV2PI)).reshape(IT, 128)
        t80 = np.empty((128, 8), np.float32)
        om8 = np.empty((128, 8), np.float32)
        for a in range(IT):
            t80[:, 2 * a] = th[a] * np.float32(INV2PI)
            t80[:, 2 * a + 1] = th[a] * np.float32(INV2PI) + np.float32(0.25)
            om8[:, 2 * a] = om[a]
            om8[:, 2 * a + 1] = om[a]
        sco0 = np.sin(TWO_PI * t80.astype(np.float64)).astype(np.float32)
        in_maps.append(
            {
                "kt": np.ascontiguousarray(Ks[sl, :][:, perm].T).astype(np.float16),
                "sc0": np.ascontiguousarray(sc0.reshape(-1)),
                "t80": np.ascontiguousarray(t80.reshape(-1)),
                "om8": np.ascontiguousarray(om8.reshape(-1)),
                "sco0": np.ascontiguousarray(sco0.reshape(-1)),
            }
        )
    res = bass_utils.run_bass_kernel_spmd(
        nc, in_maps, core_ids=list(range(M)), trace=TRACE
    )
    LAST_RESULTS = res
    out = np.concatenate(
        [res.results[c]["th_out"].reshape(128, IT).T.reshape(-1) for c in range(M)]
    )
    return out.astype(np.float32)
